# revision 25
# baseline (speedup 1.0000x reference)
"""GAT (2-layer graph attention) Trainium2 Bass kernel, 8-core row-parallel.

Strategy
--------
Shard the destination-node dimension N=8192 across 8 cores (1024 rows each).
Attention tiles are computed TRANSPOSED, [j=128 src partitions, i=1024 free].

Key identity: exp is monotone, so for z = f1[i] + f2[j],
    exp(leaky_relu(z)) = max(exp(z), exp(0.2 z))
and both branches are rank-1 separable. Dividing by the i-only factor
exp(0.2 f1[i]) (which cancels in the softmax normalization) gives
    e~[j,i] = m[j,i] * max( exp(0.8 f1[i]) * exp(f2[j]), exp(0.2 f2[j]) )
with m the 0/1 adjacency mask. Per (chunk, head) unit this is ONE stock
tensor_scalar (bf16, 4x DVE perf mode; scalars exp(f2), exp(0.2 f2) ride the
per-partition slots) plus ONE bf16 mask multiply (DVE 2x, a slice of units
offloaded to GpSimd) - no table exp over the [N, N/8] attention matrix at all.

The PV matmul needs no on-chip transposes: [ones | f2 | fts] is the
stationary operand (bf16), the masked-exp tile is the moving operand,
accumulated over all 64 source chunks in PSUM; the softmax row-sum falls out
of the same matmul via the ones column, so the division is applied to the
[hid+2, 1024] result, not the [8192, 1024] attention matrix.

The adjacency mask is transposed and cast to bf16 0/1 on the host as part of
sharding; each core streams its own [8192, 1024] column slab once per layer,
alternating between the two DMA descriptor-generation paths.
Layer-1 output h1^T is produced directly in the [feature, node] layout
layer 2 needs as its stationary operand; it is gathered on the host
between the two launches.
"""

import sys

if "/opt/trn_rl_repo" not in sys.path:
    sys.path.insert(0, "/opt/trn_rl_repo")

from contextlib import ExitStack

import ml_dtypes
import numpy as np

import concourse.bass as bass
import concourse.mybir as mybir
import concourse.tile as tile
from concourse import bacc, bass_utils
from concourse import dve_ops

F32 = mybir.dt.float32
F32R = mybir.dt.float32r
BF16 = mybir.dt.bfloat16
EXP = mybir.ActivationFunctionType.Exp
ADD = mybir.AluOpType.add
MIN = mybir.AluOpType.min
MAX = mybir.AluOpType.max
MULT = mybir.AluOpType.mult

N, F_IN, HID, NH1, NCLS = 8192, 256, 64, 4, 16
NCORES = 8
ROWS = N // NCORES  # 1024 destination rows per core
NCHUNK = N // 128  # 64 source-node chunks
ALPHA = 0.2


def _build_layer(nh, hid, fcat, Bvals, alpha, elu, repeat=1, pool_pat=None,
                 ut_bufs=3, et_bufs=4, mt_bufs=6, phases=("pro", "sweep", "ep")):
    """Build one SPMD launch (layer). Returns the compiled Bacc.

    nh:   number of heads (4 or 1)
    hid:  per-head output width (64 or 16)
    fcat: input feature dim (256 both layers)
    Bvals: per-head b_src+b_dst floats, folded into f1
    elu:  apply ELU activation to the normalized output
    """
    if pool_pat is None:
        pool_pat = (1,) if nh > 1 else (5,)
    ata = next(o for o in dve_ops.OPS if o.name == "AFFINE_THEN_ADD")
    nc = bacc.Bacc("TRN2", target_bir_lowering=False, debug=False, num_devices=1)
    kc = fcat // 128  # contraction chunks for the feature matmul
    nhp = max(nh, 4)  # f32r matmul needs moving free dim > 1; pad Wsrc
    wid = hid + 1  # [W@a_dst | W] per head
    wtot = -8 * (-(nh * wid) // 8)  # f32r moving width, padded to mult of 8
    blk = nh * (hid + 2)  # per-chunk stationary storage: nh * [ones | f2 | fts]

    # DRAM I/O ------------------------------------------------------------
    xT = nc.dram_tensor("xT", [fcat, N], BF16, kind="ExternalInput").ap()
    own_xT = nc.dram_tensor("own_xT", [fcat, ROWS], BF16, kind="ExternalInput").ap()
    Wcat = nc.dram_tensor("Wcat", [fcat, wtot], BF16, kind="ExternalInput").ap()
    Wsrc = nc.dram_tensor("Wsrc", [fcat, nhp], BF16, kind="ExternalInput").ap()
    bias_out = nc.dram_tensor("bias_out", [nh, hid], F32, kind="ExternalInput").ap()
    # pre-transposed bf16 0/1 adjacency columns for this core: mT[j, i]
    mT = nc.dram_tensor("mT", [N, ROWS], BF16, kind="ExternalInput").ap()
    outT = nc.dram_tensor("outT", [nh * hid, ROWS], F32, kind="ExternalOutput").ap()

    with tile.TileContext(nc) as tc, ExitStack() as ctx:
        const = ctx.enter_context(tc.sbuf_pool(name="const", bufs=1))

        # resident constants -------------------------------------------------
        w_t = []
        for k in range(kc):
            wk = const.tile([128, wtot], BF16, tag=f"w{k}", name=f"w{k}")
            nc.sync.dma_start(wk, Wcat[k * 128 : (k + 1) * 128, :])
            w_t.append(wk)
        ws_t = []
        for k in range(kc):
            wsk = const.tile([128, nhp], BF16, tag=f"ws{k}", name=f"ws{k}")
            nc.sync.dma_start(wsk, Wsrc[k * 128 : (k + 1) * 128, :])
            ws_t.append(wsk)
        cst = const.tile([1, 128 + hid + 2], F32, tag="cst", name="cst")
        nc.gpsimd.memset(cst, 1.0)
        nc.gpsimd.memset(cst[:, 128 : 130], 0.0)
        onesrow = const.tile([1, 128], F32R, tag="onesrow", name="onesrow")
        nc.vector.tensor_copy(onesrow, cst[:, 0:128])
        # lhsT for the reciprocal broadcast: [0, 0, 1, 1, ..., 1]
        maskh = const.tile([1, hid + 2], F32R, tag="maskh", name="maskh")
        nc.vector.tensor_copy(maskh, cst[:, 128 : 128 + hid + 2])
        ident = const.tile([128, 128], F32, tag="ident", name="ident")
        from concourse.masks import make_identity

        make_identity(nc, ident)
        bpp = []
        for h in range(nh):
            bt = const.tile([hid + 2, 1], F32, tag=f"bpp{h}", name=f"bpp{h}")
            nc.gpsimd.memset(bt, 0.0)
            nc.sync.dma_start(
                bt[2 : hid + 2, :], bias_out[h : h + 1, :].rearrange("a b -> b a")
            )
            bpp.append(bt)

        # per-chunk stationary blocks: [ones | f2 | fts] per head, bf16
        fts_all = const.tile([128, NCHUNK * blk], BF16, tag="fts", name="fts_all")
        fview = fts_all.rearrange("p (c h x) -> p c h x", c=NCHUNK, h=nh)
        nc.gpsimd.memset(fview[:, :, :, 0], 1.0)  # ones columns, strided
        E2_all = const.tile([128, NCHUNK * nh], F32, tag="e2", name="E2_all")
        E2p_all = const.tile([128, NCHUNK * nh], F32, tag="e2p", name="E2p_all")
        f1col = const.tile([128, 8 * nhp], F32, tag="f1col", name="f1col")
        r1row = []
        for h in range(nh):
            fr = const.tile([1, ROWS], F32R, tag=f"r1row{h}", name=f"r1row{h}")
            r1row.append(fr)
        r1b = []
        for h in range(nh):
            fb = const.tile([128, ROWS], BF16, tag=f"r1b{h}", name=f"r1b{h}")
            r1b.append(fb)

        def _one_pass():
            # ---- prologue: f1 -> exp(0.8 f1) own rows; fts = x @ Wcat -------
            with tc.psum_pool(name="pro", bufs=4) as pp, tc.sbuf_pool(
                name="pro_sb", bufs=2
            ) as ps:
                # own-row f1 columns first: the r1b chain is the longest pole
                sog = []
                for k in range(kc):
                    sg2 = ps.tile([128, 1024], BF16, tag=f"so{k}", name=f"so{k}")
                    (nc.gpsimd if k % 2 == 0 else nc.sync).dma_start(
                        sg2, own_xT[k * 128 : (k + 1) * 128, :]
                    )
                    sog.append(sg2)
                for r in range(8):
                    pf1 = pp.tile([128, nhp], F32, tag="f1", name="pf1")
                    for k in range(kc):
                        nc.tensor.matmul(
                            pf1,
                            lhsT=sog[k][:, r * 128 : (r + 1) * 128],
                            rhs=ws_t[k],
                            start=(k == 0),
                            stop=(k == kc - 1),
                        )
                    nc.vector.tensor_copy(f1col[:, r * nhp : (r + 1) * nhp], pf1)
                # r1row[h][i] = exp(0.8 (f1 + B)) via per-column PE transposes
                for r in range(8):
                    for h in range(nh):
                        pt = pp.tile([1, 128], F32, tag="f1", name="pt")
                        nc.tensor.transpose(
                            pt, f1col[:, r * nhp + h : r * nhp + h + 1], ident
                        )
                        with nc.allow_low_precision(reason="f32r broadcast row"):
                            nc.scalar.activation(
                                r1row[h][:, r * 128 : (r + 1) * 128],
                                pt,
                                EXP,
                                scale=1.0 - alpha,
                                bias=(1.0 - alpha) * Bvals[h],
                            )
                for h in range(nh):
                    # broadcast r1row over the 128 partitions via PE, cast bf16
                    for half in range(ROWS // 512):
                        pb = pp.tile([128, 512], F32, tag="f1", name="pb")
                        nc.tensor.matmul(
                            pb,
                            lhsT=onesrow,
                            rhs=r1row[h][:, half * 512 : (half + 1) * 512],
                            start=True,
                            stop=True,
                        )
                        nc.scalar.copy(r1b[h][:, half * 512 : (half + 1) * 512], pb)

                for jc in range(NCHUNK):
                    jg, jr = divmod(jc, 8)
                    if jr == 0:
                        sqg = []
                        for k in range(kc):
                            sg = ps.tile([128, 1024], BF16, tag=f"sq{k}", name=f"sq{k}")
                            # split the x stream across both DGE paths
                            (nc.gpsimd if (jg + k) % 2 == 0 else nc.sync).dma_start(
                                sg,
                                xT[k * 128 : (k + 1) * 128, jg * 1024 : (jg + 1) * 1024],
                            )
                            sqg.append(sg)
                    pf = pp.tile([128, wtot], F32, tag="ps", name="pf")
                    for k in range(kc):
                        nc.tensor.matmul(
                            pf,
                            lhsT=sqg[k][:, jr * 128 : (jr + 1) * 128],
                            rhs=w_t[k],
                            start=(k == 0),
                            stop=(k == kc - 1),
                        )
                    # [f2 | fts] cols -> block cols 1..hid+2 in one copy,
                    # rotated across ACT/DVE/Pool so no single engine serializes
                    cpeng = (nc.scalar.copy, nc.vector.tensor_copy)[jc % 2]
                    cpeng(
                        fview[:, jc, :, 1 : hid + 2],
                        pf[:, 0 : nh * wid].rearrange("p (h x) -> p h x", h=nh),
                    )
                    if jr == 7:
                        # per-group exp so the sweep can start after group 0
                        nc.scalar.activation(
                            E2_all[:, jg * 8 * nh : (jg + 1) * 8 * nh],
                            fview[:, jg * 8 : (jg + 1) * 8, :, 1],
                            EXP,
                            scale=1.0,
                        )
                        nc.scalar.activation(
                            E2p_all[:, jg * 8 * nh : (jg + 1) * 8 * nh],
                            fview[:, jg * 8 : (jg + 1) * 8, :, 1],
                            EXP,
                            scale=alpha,
                        )

            if "sweep" not in phases:
                return
            # ---- attention sweep over source chunks -------------------------
            with tc.psum_pool(name="acc", bufs=1) as ap_, tc.sbuf_pool(
                name="sw", bufs=3
            ) as sw, tc.sbuf_pool(name="ep", bufs=2) as ep:
                accs = []
                for i in range(2 * nh):
                    a = ap_.tile([hid + 2, 512], F32, tag=f"acc{i}", name=f"acc{i}", bufs=1)
                    accs.append(a)
                pend = []  # GpSimd-path matmuls deferred by one chunk
                for jc in range(NCHUNK):
                    mt = sw.tile([128, ROWS], BF16, tag="mt", name="mt", bufs=mt_bufs)
                    # alternate DGE paths so neither descriptor engine serializes
                    (nc.gpsimd if jc % 2 == 0 else nc.sync).dma_start(
                        mt, mT[jc * 128 : (jc + 1) * 128, :]
                    )
                    # GpSimd takes the tail heads on interior chunks, so chunk 0
                    # (start) and the last chunk (stop) stay on the DVE path
                    if 0 < jc < NCHUNK - 1 and pool_pat:
                        if nh == 1:
                            mod = pool_pat[0] if pool_pat else 0
                            npool = 1 if (mod and jc % mod == 2) else 0
                        else:
                            npool = min(pool_pat[jc % len(pool_pat)], nh - 1)
                    else:
                        npool = 0
                    ndve = nh - npool
                    u4 = sw.tile([128, nh * ROWS], BF16, tag="u4", name="u4", bufs=ut_bufs)
                    e4 = sw.tile([128, nh * ROWS], BF16, tag="e4", name="e4", bufs=et_bufs)
                    for h in range(nh):
                        u = jc * nh + h
                        # u = max(exp(0.8 f1)[i] * exp(f2)[j], exp(0.2 f2)[j])
                        nc.vector.tensor_scalar(
                            u4[:, h * ROWS : (h + 1) * ROWS],
                            r1b[h],
                            E2_all[:, u : u + 1],
                            E2p_all[:, u : u + 1],
                            op0=MULT,
                            op1=MAX,
                        )
                    # masked exp: one batched DVE multiply for the DVE heads
                    if ndve:
                        mtb = mt.unsqueeze(1).broadcast_to([128, ndve, ROWS])
                        nc.vector.tensor_tensor(
                            e4[:, : ndve * ROWS].rearrange("p (h i) -> p h i", h=ndve),
                            u4[:, : ndve * ROWS].rearrange("p (h i) -> p h i", h=ndve),
                            mtb,
                            op=MULT,
                        )
                    for h in range(ndve, nh):
                        nc.gpsimd.tensor_mul(
                            e4[:, h * ROWS : (h + 1) * ROWS],
                            u4[:, h * ROWS : (h + 1) * ROWS],
                            mt,
                        )
                    # deferred GpSimd matmuls from the previous chunk first
                    for pacc, plhs, prhs in pend:
                        nc.tensor.matmul(pacc, lhsT=plhs, rhs=prhs, start=False, stop=False)
                    pend = []
                    for h in range(nh):
                        lhs = fview[:, jc, h, :]
                        for half in range(2):
                            rhs = e4[:, h * ROWS + half * 512 : h * ROWS + (half + 1) * 512]
                            if h >= ndve:
                                pend.append((accs[2 * h + half], lhs, rhs))
                            else:
                                nc.tensor.matmul(
                                    accs[2 * h + half],
                                    lhsT=lhs,
                                    rhs=rhs,
                                    start=(jc == 0),
                                    stop=(jc == NCHUNK - 1),
                                )

                # ---- epilogue: normalize (+bias, +ELU), store h^T -----------
                for h in range(nh if "ep" in phases else 0):
                    v = ep.tile([hid + 2, ROWS], F32, tag="v", name="v")
                    for half in range(2):
                        nc.scalar.copy(
                            v[:, half * 512 : (half + 1) * 512], accs[2 * h + half]
                        )
                    rc = ep.tile([1, ROWS], F32R, tag="rc", name="rc")
                    with nc.allow_low_precision(reason="f32r out of reciprocal"):
                        nc.vector.reciprocal(rc, v[0:1, :])
                    t = ep.tile([hid + 2, ROWS], F32, tag="t", name="t")
                    for half in range(2):
                        pb2 = ap_.tile(
                            [hid + 2, 512], F32, tag=f"acc{2 * h + half}", name="pb2", bufs=1
                        )
                        nc.tensor.matmul(
                            pb2,
                            lhsT=maskh,
                            rhs=rc[:, half * 512 : (half + 1) * 512],
                            start=True,
                            stop=True,
                        )
                        nc.vector.tensor_tensor(
                            t[:, half * 512 : (half + 1) * 512],
                            v[:, half * 512 : (half + 1) * 512],
                            pb2,
                            op=MULT,
                        )
                    # rows 0-1 carry harmless junk through the tail ops
                    if elu:
                        m_ = ep.tile([hid + 2, ROWS], F32, tag="m_", name="m_")
                        nc.vector.tensor_scalar(m_, t, bpp[h], 0.0, op0=ADD, op1=MIN)
                        r_ = ep.tile([hid + 2, ROWS], F32, tag="r_", name="r_")
                        nc.vector.tensor_scalar(r_, t, bpp[h], 0.0, op0=ADD, op1=MAX)
                        e2 = ep.tile([hid + 2, ROWS], F32, tag="e2t", name="e2t")
                        nc.scalar.activation(e2, m_, EXP)
                        o_ = ep.tile([hid + 2, ROWS], F32, tag="o_", name="o_")
                        nc.vector._custom_dve(ata, out=o_, in0=e2, in1=r_, s0=1.0, s1=-1.0)
                    else:
                        o_ = ep.tile([hid + 2, ROWS], F32, tag="o_", name="o_")
                        nc.vector.tensor_scalar(o_, t, bpp[h], None, op0=ADD)
                    nc.sync.dma_start(outT[h * hid : (h + 1) * hid, :], o_[2 : hid + 2, :])

        for _rep in range(repeat):
            _one_pass()

    nc.compile()
    return nc


_BUILD_CACHE: dict = {}


def _get_layer(key, *args):
    if key not in _BUILD_CACHE:
        _BUILD_CACHE[key] = _build_layer(*args)
    return _BUILD_CACHE[key]


def _make_wcat(W, a_dst, nh):
    """Per head [W @ a_dst | W], concat over heads, zero-padded to mult of 8."""
    cat = np.concatenate(
        [np.concatenate([W[h] @ a_dst[h], W[h]], axis=1) for h in range(nh)],
        axis=1,
    ).astype(np.float32)
    pad = -8 * (-cat.shape[1] // 8) - cat.shape[1]
    if pad:
        cat = np.concatenate([cat, np.zeros((cat.shape[0], pad), np.float32)], axis=1)
    return cat.astype(ml_dtypes.bfloat16)


def kernel(
    seq,
    bias_mat,
    W1,
    a1_src,
    a1_dst,
    b1_src,
    b1_dst,
    bias1,
    W2,
    a2_src,
    a2_dst,
    b2_src,
    b2_dst,
    bias2,
):
    seq = np.asarray(seq, np.float32)
    bias_mat = np.asarray(bias_mat, np.float32)
    W1, W2 = np.asarray(W1, np.float32), np.asarray(W2, np.float32)
    a1_src, a1_dst = np.asarray(a1_src, np.float32), np.asarray(a1_dst, np.float32)
    a2_src, a2_dst = np.asarray(a2_src, np.float32), np.asarray(a2_dst, np.float32)
    bias1, bias2 = np.asarray(bias1, np.float32), np.asarray(bias2, np.float32)

    x = seq[0]  # [N, F_IN]
    xT = np.ascontiguousarray(x.T).astype(ml_dtypes.bfloat16)  # [F_IN, N]
    # per-core transposed 0/1 bf16 mask slabs: mT_c[j, i] = (bias[c*ROWS+i, j] == 0)
    mTs = [
        np.ascontiguousarray(
            (bias_mat[0, c * ROWS : (c + 1) * ROWS, :] == 0.0).T
        ).astype(ml_dtypes.bfloat16)
        for c in range(NCORES)
    ]
    W1cat = _make_wcat(W1, a1_dst, NH1)
    W1s = np.concatenate([W1[h] @ a1_src[h] for h in range(NH1)], axis=1).astype(
        ml_dtypes.bfloat16
    )  # [256, 4]
    B1 = tuple(float(b1_src[h, 0] + b1_dst[h, 0]) for h in range(NH1))

    nc1 = _get_layer(("L1", B1), NH1, HID, F_IN, B1, ALPHA, True)
    in_maps = []
    for c in range(NCORES):
        in_maps.append(
            {
                "xT": xT,
                "own_xT": np.ascontiguousarray(xT[:, c * ROWS : (c + 1) * ROWS]),
                "Wcat": W1cat,
                "Wsrc": W1s,
                "bias_out": bias1,
                "mT": mTs[c],
            }
        )
    res1 = bass_utils.run_bass_kernel_spmd(nc1, in_maps, core_ids=list(range(NCORES)))
    h1T = np.concatenate([r["outT"] for r in res1.results], axis=1).astype(
        ml_dtypes.bfloat16
    )  # [256, 8192]

    W2cat = _make_wcat(W2, a2_dst, 1)
    W2s = np.concatenate(
        [W2[0] @ a2_src[0], np.zeros((NH1 * HID, 3), np.float32)], axis=1
    ).astype(ml_dtypes.bfloat16)
    B2 = (float(b2_src[0, 0] + b2_dst[0, 0]),)

    nc2 = _get_layer(("L2", B2), 1, NCLS, NH1 * HID, B2, ALPHA, False)
    in_maps2 = []
    for c in range(NCORES):
        in_maps2.append(
            {
                "xT": h1T,
                "own_xT": np.ascontiguousarray(h1T[:, c * ROWS : (c + 1) * ROWS]),
                "Wcat": W2cat,
                "Wsrc": W2s,
                "bias_out": bias2,
                "mT": mTs[c],
            }
        )
    res2 = bass_utils.run_bass_kernel_spmd(nc2, in_maps2, core_ids=list(range(NCORES)))
    outT = np.concatenate([r["outT"] for r in res2.results], axis=1)  # [16, 8192]
    return np.ascontiguousarray(outT.T)[None].astype(np.float32)  # [1, 8192, 16]


# revision 26
# speedup vs baseline: 1.4697x; 1.4697x over previous
"""GAT (2-layer graph attention) Trainium2 Bass kernel, 8-core row-parallel.

Strategy
--------
Shard the destination-node dimension N=8192 across 8 cores (1024 rows each).
Attention tiles are computed TRANSPOSED, [j=128 src partitions, i=1024 free].

Key identity: exp is monotone, so for z = f1[i] + f2[j],
    exp(leaky_relu(z)) = max(exp(z), exp(0.2 z))
and both branches are rank-1 separable. Dividing by the i-only factor
exp(0.2 f1[i]) (which cancels in the softmax normalization) gives
    e~[j,i] = m[j,i] * max( exp(0.8 f1[i]) * exp(f2[j]), exp(0.2 f2[j]) )
with m the 0/1 adjacency mask. Per (chunk, head) unit this is ONE stock
tensor_scalar (bf16, 4x DVE perf mode; scalars exp(f2), exp(0.2 f2) ride the
per-partition slots) plus ONE bf16 mask multiply (DVE 2x, a slice of units
offloaded to GpSimd) - no table exp over the [N, N/8] attention matrix at all.

The PV matmul needs no on-chip transposes: [ones | f2 | fts] is the
stationary operand (bf16), the masked-exp tile is the moving operand,
accumulated over all 64 source chunks in PSUM; the softmax row-sum falls out
of the same matmul via the ones column, so the division is applied to the
[hid+2, 1024] result, not the [8192, 1024] attention matrix.

The adjacency mask is transposed and cast to bf16 0/1 on the host as part of
sharding; each core streams its own [8192, 1024] column slab once per layer,
alternating between the two DMA descriptor-generation paths.
Layer-1 output h1^T is produced directly in the [feature, node] layout
layer 2 needs as its stationary operand; it is gathered on the host
between the two launches.
"""

import sys

if "/opt/trn_rl_repo" not in sys.path:
    sys.path.insert(0, "/opt/trn_rl_repo")

from contextlib import ExitStack

import ml_dtypes
import numpy as np

import concourse.bass as bass
import concourse.mybir as mybir
import concourse.tile as tile
from concourse import bacc, bass_utils
from concourse import dve_ops

F32 = mybir.dt.float32
F32R = mybir.dt.float32r
BF16 = mybir.dt.bfloat16
EXP = mybir.ActivationFunctionType.Exp
ADD = mybir.AluOpType.add
MIN = mybir.AluOpType.min
MAX = mybir.AluOpType.max
MULT = mybir.AluOpType.mult

N, F_IN, HID, NH1, NCLS = 8192, 256, 64, 4, 16
NCORES = 8
ROWS = N // NCORES  # 1024 destination rows per core
NCHUNK = N // 128  # 64 source-node chunks
ALPHA = 0.2


def _build_layer(nh, hid, fcat, Bvals, alpha, elu, repeat=1, pool_pat=None,
                 ut_bufs=3, et_bufs=4, mt_bufs=6, phases=("pro", "sweep", "ep")):
    """Build one SPMD launch (layer). Returns the compiled Bacc.

    nh:   number of heads (4 or 1)
    hid:  per-head output width (64 or 16)
    fcat: input feature dim (256 both layers)
    Bvals: per-head b_src+b_dst floats, folded into f1
    elu:  apply ELU activation to the normalized output
    """
    if pool_pat is None:
        pool_pat = (1,) if nh > 1 else (6,)
    ata = next(o for o in dve_ops.OPS if o.name == "AFFINE_THEN_ADD")
    nc = bacc.Bacc("TRN2", target_bir_lowering=False, debug=False, num_devices=1)
    kc = fcat // 128  # contraction chunks for the feature matmul
    nhp = max(nh, 4)  # f32r matmul needs moving free dim > 1; pad Wsrc
    wid = hid + 1  # [W@a_dst | W] per head
    wtot = -8 * (-(nh * wid) // 8)  # f32r moving width, padded to mult of 8
    blk = nh * (hid + 2)  # per-chunk stationary storage: nh * [ones | f2 | fts]

    # DRAM I/O ------------------------------------------------------------
    xT = nc.dram_tensor("xT", [fcat, N], BF16, kind="ExternalInput").ap()
    own_xT = nc.dram_tensor("own_xT", [fcat, ROWS], BF16, kind="ExternalInput").ap()
    Wcat = nc.dram_tensor("Wcat", [fcat, wtot], BF16, kind="ExternalInput").ap()
    Wsrc = nc.dram_tensor("Wsrc", [fcat, nhp], BF16, kind="ExternalInput").ap()
    bias_out = nc.dram_tensor("bias_out", [nh, hid], F32, kind="ExternalInput").ap()
    # pre-transposed bf16 0/1 adjacency columns for this core: mT[j, i]
    mT = nc.dram_tensor("mT", [N, ROWS], BF16, kind="ExternalInput").ap()
    outT = nc.dram_tensor("outT", [nh * hid, ROWS], F32, kind="ExternalOutput").ap()

    with tile.TileContext(nc) as tc, ExitStack() as ctx:
        const = ctx.enter_context(tc.sbuf_pool(name="const", bufs=1))

        # resident constants -------------------------------------------------
        w_t = []
        for k in range(kc):
            wk = const.tile([128, wtot], BF16, tag=f"w{k}", name=f"w{k}")
            nc.sync.dma_start(wk, Wcat[k * 128 : (k + 1) * 128, :])
            w_t.append(wk)
        ws_t = []
        for k in range(kc):
            wsk = const.tile([128, nhp], BF16, tag=f"ws{k}", name=f"ws{k}")
            nc.sync.dma_start(wsk, Wsrc[k * 128 : (k + 1) * 128, :])
            ws_t.append(wsk)
        cst = const.tile([1, 128 + hid + 2], F32, tag="cst", name="cst")
        nc.gpsimd.memset(cst, 1.0)
        nc.gpsimd.memset(cst[:, 128 : 130], 0.0)
        onesrow = const.tile([1, 128], F32R, tag="onesrow", name="onesrow")
        nc.vector.tensor_copy(onesrow, cst[:, 0:128])
        # lhsT for the reciprocal broadcast: [0, 0, 1, 1, ..., 1]
        maskh = const.tile([1, hid + 2], F32R, tag="maskh", name="maskh")
        nc.vector.tensor_copy(maskh, cst[:, 128 : 128 + hid + 2])
        ident = const.tile([128, 128], F32, tag="ident", name="ident")
        from concourse.masks import make_identity

        make_identity(nc, ident)
        bpp = []
        for h in range(nh):
            bt = const.tile([hid + 2, 1], F32, tag=f"bpp{h}", name=f"bpp{h}")
            nc.gpsimd.memset(bt, 0.0)
            nc.sync.dma_start(
                bt[2 : hid + 2, :], bias_out[h : h + 1, :].rearrange("a b -> b a")
            )
            bpp.append(bt)

        # per-chunk stationary blocks: [ones | f2 | fts] per head, bf16
        fts_all = const.tile([128, NCHUNK * blk], BF16, tag="fts", name="fts_all")
        fview = fts_all.rearrange("p (c h x) -> p c h x", c=NCHUNK, h=nh)
        nc.gpsimd.memset(fview[:, :, :, 0], 1.0)  # ones columns, strided
        E2_all = const.tile([128, NCHUNK * nh], F32, tag="e2", name="E2_all")
        E2p_all = const.tile([128, NCHUNK * nh], F32, tag="e2p", name="E2p_all")
        f1col = const.tile([128, 8 * nhp], F32, tag="f1col", name="f1col")
        r1row = []
        for h in range(nh):
            fr = const.tile([1, ROWS], F32R, tag=f"r1row{h}", name=f"r1row{h}")
            r1row.append(fr)
        r1b = []
        for h in range(nh):
            fb = const.tile([128, ROWS], BF16, tag=f"r1b{h}", name=f"r1b{h}")
            r1b.append(fb)

        def _one_pass():
            # ---- prologue: f1 -> exp(0.8 f1) own rows; fts = x @ Wcat -------
            with tc.psum_pool(name="pro", bufs=4) as pp, tc.sbuf_pool(
                name="pro_sb", bufs=2
            ) as ps:
                # own-row f1 columns first: the r1b chain is the longest pole
                sog = []
                for k in range(kc):
                    sg2 = ps.tile([128, 1024], BF16, tag=f"so{k}", name=f"so{k}")
                    (nc.gpsimd if k % 2 == 0 else nc.sync).dma_start(
                        sg2, own_xT[k * 128 : (k + 1) * 128, :]
                    )
                    sog.append(sg2)
                for r in range(8):
                    pf1 = pp.tile([128, nhp], F32, tag="f1", name="pf1")
                    for k in range(kc):
                        nc.tensor.matmul(
                            pf1,
                            lhsT=sog[k][:, r * 128 : (r + 1) * 128],
                            rhs=ws_t[k],
                            start=(k == 0),
                            stop=(k == kc - 1),
                        )
                    nc.vector.tensor_copy(f1col[:, r * nhp : (r + 1) * nhp], pf1)
                # r1row[h][i] = exp(0.8 (f1 + B)) via per-column PE transposes
                for r in range(8):
                    for h in range(nh):
                        pt = pp.tile([1, 128], F32, tag="f1", name="pt")
                        nc.tensor.transpose(
                            pt, f1col[:, r * nhp + h : r * nhp + h + 1], ident
                        )
                        with nc.allow_low_precision(reason="f32r broadcast row"):
                            nc.scalar.activation(
                                r1row[h][:, r * 128 : (r + 1) * 128],
                                pt,
                                EXP,
                                scale=1.0 - alpha,
                                bias=(1.0 - alpha) * Bvals[h],
                            )
                for h in range(nh):
                    # broadcast r1row over the 128 partitions via PE, cast bf16
                    for half in range(ROWS // 512):
                        pb = pp.tile([128, 512], F32, tag="f1", name="pb")
                        nc.tensor.matmul(
                            pb,
                            lhsT=onesrow,
                            rhs=r1row[h][:, half * 512 : (half + 1) * 512],
                            start=True,
                            stop=True,
                        )
                        nc.scalar.copy(r1b[h][:, half * 512 : (half + 1) * 512], pb)

                for jc in range(NCHUNK):
                    jg, jr = divmod(jc, 8)
                    if jr == 0:
                        sqg = []
                        for k in range(kc):
                            sg = ps.tile([128, 1024], BF16, tag=f"sq{k}", name=f"sq{k}")
                            # split the x stream across both DGE paths
                            (nc.gpsimd if (jg + k) % 2 == 0 else nc.sync).dma_start(
                                sg,
                                xT[k * 128 : (k + 1) * 128, jg * 1024 : (jg + 1) * 1024],
                            )
                            sqg.append(sg)
                    pf = pp.tile([128, wtot], F32, tag="ps", name="pf")
                    for k in range(kc):
                        nc.tensor.matmul(
                            pf,
                            lhsT=sqg[k][:, jr * 128 : (jr + 1) * 128],
                            rhs=w_t[k],
                            start=(k == 0),
                            stop=(k == kc - 1),
                        )
                    # [f2 | fts] cols -> block cols 1..hid+2 in one copy,
                    # rotated across ACT/DVE/Pool so no single engine serializes
                    cpeng = (nc.scalar.copy, nc.vector.tensor_copy)[jc % 2]
                    cpeng(
                        fview[:, jc, :, 1 : hid + 2],
                        pf[:, 0 : nh * wid].rearrange("p (h x) -> p h x", h=nh),
                    )
                    if jr == 7:
                        # per-group exp so the sweep can start after group 0
                        nc.scalar.activation(
                            E2_all[:, jg * 8 * nh : (jg + 1) * 8 * nh],
                            fview[:, jg * 8 : (jg + 1) * 8, :, 1],
                            EXP,
                            scale=1.0,
                        )
                        nc.scalar.activation(
                            E2p_all[:, jg * 8 * nh : (jg + 1) * 8 * nh],
                            fview[:, jg * 8 : (jg + 1) * 8, :, 1],
                            EXP,
                            scale=alpha,
                        )

            if "sweep" not in phases:
                return
            # ---- attention sweep over source chunks -------------------------
            with tc.psum_pool(name="acc", bufs=1) as ap_, tc.sbuf_pool(
                name="sw", bufs=3
            ) as sw, tc.sbuf_pool(name="ep", bufs=2) as ep:
                accs = []
                for i in range(2 * nh):
                    a = ap_.tile([hid + 2, 512], F32, tag=f"acc{i}", name=f"acc{i}", bufs=1)
                    accs.append(a)
                pend = []  # GpSimd-path matmuls deferred by one chunk
                for jc in range(NCHUNK):
                    mt = sw.tile([128, ROWS], BF16, tag="mt", name="mt", bufs=mt_bufs)
                    # alternate DGE paths so neither descriptor engine serializes
                    (nc.gpsimd if jc % 2 == 0 else nc.sync).dma_start(
                        mt, mT[jc * 128 : (jc + 1) * 128, :]
                    )
                    # GpSimd takes the tail heads on interior chunks, so chunk 0
                    # (start) and the last chunk (stop) stay on the DVE path
                    if 0 < jc < NCHUNK - 1 and pool_pat:
                        if nh == 1:
                            mod = pool_pat[0] if pool_pat else 0
                            npool = 1 if (mod and jc % mod == 2) else 0
                        else:
                            npool = min(pool_pat[jc % len(pool_pat)], nh - 1)
                    else:
                        npool = 0
                    ndve = nh - npool
                    u4 = sw.tile([128, nh * ROWS], BF16, tag="u4", name="u4", bufs=ut_bufs)
                    e4 = sw.tile([128, nh * ROWS], BF16, tag="e4", name="e4", bufs=et_bufs)
                    for h in range(nh):
                        u = jc * nh + h
                        # u = max(exp(0.8 f1)[i] * exp(f2)[j], exp(0.2 f2)[j])
                        nc.vector.tensor_scalar(
                            u4[:, h * ROWS : (h + 1) * ROWS],
                            r1b[h],
                            E2_all[:, u : u + 1],
                            E2p_all[:, u : u + 1],
                            op0=MULT,
                            op1=MAX,
                        )
                    # masked exp: one batched DVE multiply for the DVE heads
                    if ndve:
                        mtb = mt.unsqueeze(1).broadcast_to([128, ndve, ROWS])
                        nc.vector.tensor_tensor(
                            e4[:, : ndve * ROWS].rearrange("p (h i) -> p h i", h=ndve),
                            u4[:, : ndve * ROWS].rearrange("p (h i) -> p h i", h=ndve),
                            mtb,
                            op=MULT,
                        )
                    for h in range(ndve, nh):
                        nc.gpsimd.tensor_mul(
                            e4[:, h * ROWS : (h + 1) * ROWS],
                            u4[:, h * ROWS : (h + 1) * ROWS],
                            mt,
                        )
                    # deferred GpSimd matmuls from the previous chunk first
                    for pacc, plhs, prhs in pend:
                        nc.tensor.matmul(pacc, lhsT=plhs, rhs=prhs, start=False, stop=False)
                    pend = []
                    for h in range(nh):
                        lhs = fview[:, jc, h, :]
                        for half in range(2):
                            rhs = e4[:, h * ROWS + half * 512 : h * ROWS + (half + 1) * 512]
                            if h >= ndve:
                                pend.append((accs[2 * h + half], lhs, rhs))
                            else:
                                nc.tensor.matmul(
                                    accs[2 * h + half],
                                    lhsT=lhs,
                                    rhs=rhs,
                                    start=(jc == 0),
                                    stop=(jc == NCHUNK - 1),
                                )

                # ---- epilogue: normalize (+bias, +ELU), store h^T -----------
                for h in range(nh if "ep" in phases else 0):
                    v = ep.tile([hid + 2, ROWS], F32, tag="v", name="v")
                    for half in range(2):
                        nc.scalar.copy(
                            v[:, half * 512 : (half + 1) * 512], accs[2 * h + half]
                        )
                    rc = ep.tile([1, ROWS], F32R, tag="rc", name="rc")
                    with nc.allow_low_precision(reason="f32r out of reciprocal"):
                        nc.vector.reciprocal(rc, v[0:1, :])
                    t = ep.tile([hid + 2, ROWS], F32, tag="t", name="t")
                    for half in range(2):
                        pb2 = ap_.tile(
                            [hid + 2, 512], F32, tag=f"acc{2 * h + half}", name="pb2", bufs=1
                        )
                        nc.tensor.matmul(
                            pb2,
                            lhsT=maskh,
                            rhs=rc[:, half * 512 : (half + 1) * 512],
                            start=True,
                            stop=True,
                        )
                        nc.vector.tensor_tensor(
                            t[:, half * 512 : (half + 1) * 512],
                            v[:, half * 512 : (half + 1) * 512],
                            pb2,
                            op=MULT,
                        )
                    # rows 0-1 carry harmless junk through the tail ops
                    if elu:
                        m_ = ep.tile([hid + 2, ROWS], F32, tag="m_", name="m_")
                        nc.vector.tensor_scalar(m_, t, bpp[h], 0.0, op0=ADD, op1=MIN)
                        r_ = ep.tile([hid + 2, ROWS], F32, tag="r_", name="r_")
                        nc.vector.tensor_scalar(r_, t, bpp[h], 0.0, op0=ADD, op1=MAX)
                        e2 = ep.tile([hid + 2, ROWS], F32, tag="e2t", name="e2t")
                        nc.scalar.activation(e2, m_, EXP)
                        o_ = ep.tile([hid + 2, ROWS], F32, tag="o_", name="o_")
                        nc.vector._custom_dve(ata, out=o_, in0=e2, in1=r_, s0=1.0, s1=-1.0)
                    else:
                        o_ = ep.tile([hid + 2, ROWS], F32, tag="o_", name="o_")
                        nc.vector.tensor_scalar(o_, t, bpp[h], None, op0=ADD)
                    nc.sync.dma_start(outT[h * hid : (h + 1) * hid, :], o_[2 : hid + 2, :])

        for _rep in range(repeat):
            _one_pass()

    nc.compile()
    return nc


_BUILD_CACHE: dict = {}


def _get_layer(key, *args):
    if key not in _BUILD_CACHE:
        _BUILD_CACHE[key] = _build_layer(*args)
    return _BUILD_CACHE[key]


def _make_wcat(W, a_dst, nh):
    """Per head [W @ a_dst | W], concat over heads, zero-padded to mult of 8."""
    cat = np.concatenate(
        [np.concatenate([W[h] @ a_dst[h], W[h]], axis=1) for h in range(nh)],
        axis=1,
    ).astype(np.float32)
    pad = -8 * (-cat.shape[1] // 8) - cat.shape[1]
    if pad:
        cat = np.concatenate([cat, np.zeros((cat.shape[0], pad), np.float32)], axis=1)
    return cat.astype(ml_dtypes.bfloat16)


def kernel(
    seq,
    bias_mat,
    W1,
    a1_src,
    a1_dst,
    b1_src,
    b1_dst,
    bias1,
    W2,
    a2_src,
    a2_dst,
    b2_src,
    b2_dst,
    bias2,
):
    seq = np.asarray(seq, np.float32)
    bias_mat = np.asarray(bias_mat, np.float32)
    W1, W2 = np.asarray(W1, np.float32), np.asarray(W2, np.float32)
    a1_src, a1_dst = np.asarray(a1_src, np.float32), np.asarray(a1_dst, np.float32)
    a2_src, a2_dst = np.asarray(a2_src, np.float32), np.asarray(a2_dst, np.float32)
    bias1, bias2 = np.asarray(bias1, np.float32), np.asarray(bias2, np.float32)

    x = seq[0]  # [N, F_IN]
    xT = np.ascontiguousarray(x.T).astype(ml_dtypes.bfloat16)  # [F_IN, N]
    # per-core transposed 0/1 bf16 mask slabs: mT_c[j, i] = (bias[c*ROWS+i, j] == 0)
    mTs = [
        np.ascontiguousarray(
            (bias_mat[0, c * ROWS : (c + 1) * ROWS, :] == 0.0).T
        ).astype(ml_dtypes.bfloat16)
        for c in range(NCORES)
    ]
    W1cat = _make_wcat(W1, a1_dst, NH1)
    W1s = np.concatenate([W1[h] @ a1_src[h] for h in range(NH1)], axis=1).astype(
        ml_dtypes.bfloat16
    )  # [256, 4]
    B1 = tuple(float(b1_src[h, 0] + b1_dst[h, 0]) for h in range(NH1))

    nc1 = _get_layer(("L1", B1), NH1, HID, F_IN, B1, ALPHA, True)
    in_maps = []
    for c in range(NCORES):
        in_maps.append(
            {
                "xT": xT,
                "own_xT": np.ascontiguousarray(xT[:, c * ROWS : (c + 1) * ROWS]),
                "Wcat": W1cat,
                "Wsrc": W1s,
                "bias_out": bias1,
                "mT": mTs[c],
            }
        )
    res1 = bass_utils.run_bass_kernel_spmd(nc1, in_maps, core_ids=list(range(NCORES)))
    h1T = np.concatenate([r["outT"] for r in res1.results], axis=1).astype(
        ml_dtypes.bfloat16
    )  # [256, 8192]

    W2cat = _make_wcat(W2, a2_dst, 1)
    W2s = np.concatenate(
        [W2[0] @ a2_src[0], np.zeros((NH1 * HID, 3), np.float32)], axis=1
    ).astype(ml_dtypes.bfloat16)
    B2 = (float(b2_src[0, 0] + b2_dst[0, 0]),)

    nc2 = _get_layer(("L2", B2), 1, NCLS, NH1 * HID, B2, ALPHA, False)
    in_maps2 = []
    for c in range(NCORES):
        in_maps2.append(
            {
                "xT": h1T,
                "own_xT": np.ascontiguousarray(h1T[:, c * ROWS : (c + 1) * ROWS]),
                "Wcat": W2cat,
                "Wsrc": W2s,
                "bias_out": bias2,
                "mT": mTs[c],
            }
        )
    res2 = bass_utils.run_bass_kernel_spmd(nc2, in_maps2, core_ids=list(range(NCORES)))
    outT = np.concatenate([r["outT"] for r in res2.results], axis=1)  # [16, 8192]
    return np.ascontiguousarray(outT.T)[None].astype(np.float32)  # [1, 8192, 16]


# revision 27
# speedup vs baseline: 1.6010x; 1.0894x over previous
"""GAT (2-layer graph attention) Trainium2 Bass kernel, 8-core row-parallel.

Strategy
--------
Shard the destination-node dimension N=8192 across 8 cores (1024 rows each).
Attention tiles are computed TRANSPOSED, [j=128 src partitions, i=1024 free].

Key identity: exp is monotone, so for z = f1[i] + f2[j],
    exp(leaky_relu(z)) = max(exp(z), exp(0.2 z))
and both branches are rank-1 separable. Dividing by the i-only factor
exp(0.2 f1[i]) (which cancels in the softmax normalization) gives
    e~[j,i] = m[j,i] * max( exp(0.8 f1[i]) * exp(f2[j]), exp(0.2 f2[j]) )
with m the 0/1 adjacency mask. Per (chunk, head) unit this is ONE stock
tensor_scalar (bf16, 4x DVE perf mode; scalars exp(f2), exp(0.2 f2) ride the
per-partition slots) plus ONE bf16 mask multiply (DVE 2x, a slice of units
offloaded to GpSimd) - no table exp over the [N, N/8] attention matrix at all.

The PV matmul needs no on-chip transposes: [ones | f2 | fts] is the
stationary operand (bf16), the masked-exp tile is the moving operand,
accumulated over all 64 source chunks in PSUM; the softmax row-sum falls out
of the same matmul via the ones column, so the division is applied to the
[hid+2, 1024] result, not the [8192, 1024] attention matrix.

The adjacency mask is transposed and cast to bf16 0/1 on the host as part of
sharding; each core streams its own [8192, 1024] column slab once per layer,
alternating between the two DMA descriptor-generation paths.
Layer-1 output h1^T is produced directly in the [feature, node] layout
layer 2 needs as its stationary operand; it is gathered on the host
between the two launches.
"""

import sys

if "/opt/trn_rl_repo" not in sys.path:
    sys.path.insert(0, "/opt/trn_rl_repo")

from contextlib import ExitStack

import ml_dtypes
import numpy as np

import concourse.bass as bass
import concourse.mybir as mybir
import concourse.tile as tile
from concourse import bacc, bass_utils
from concourse import dve_ops

F32 = mybir.dt.float32
F32R = mybir.dt.float32r
BF16 = mybir.dt.bfloat16
EXP = mybir.ActivationFunctionType.Exp
RELU = mybir.ActivationFunctionType.Relu
ADD = mybir.AluOpType.add
MIN = mybir.AluOpType.min
MAX = mybir.AluOpType.max
MULT = mybir.AluOpType.mult

N, F_IN, HID, NH1, NCLS = 8192, 256, 64, 4, 16
NCORES = 8
ROWS = N // NCORES  # 1024 destination rows per core
NCHUNK = N // 128  # 64 source-node chunks
ALPHA = 0.2


def _build_layer(nh, hid, fcat, Bvals, alpha, elu, repeat=1, pool_pat=None,
                 ut_bufs=3, et_bufs=4, mt_bufs=6, act_assist=None,
                 phases=("pro", "sweep", "ep")):
    """Build one SPMD launch (layer). Returns the compiled Bacc.

    nh:   number of heads (4 or 1)
    hid:  per-head output width (64 or 16)
    fcat: input feature dim (256 both layers)
    Bvals: per-head b_src+b_dst floats, folded into f1
    elu:  apply ELU activation to the normalized output
    """
    if pool_pat is None:
        pool_pat = (1,) if nh > 1 else (6,)
    if act_assist is None:
        act_assist = nh > 1
    ata = next(o for o in dve_ops.OPS if o.name == "AFFINE_THEN_ADD")
    nc = bacc.Bacc("TRN2", target_bir_lowering=False, debug=False, num_devices=1)
    kc = fcat // 128  # contraction chunks for the feature matmul
    nhp = max(nh, 4)  # f32r matmul needs moving free dim > 1; pad Wsrc
    wid = hid + 1  # [W@a_dst | W] per head
    wtot = -8 * (-(nh * wid) // 8)  # f32r moving width, padded to mult of 8
    blk = nh * (hid + 2)  # per-chunk stationary storage: nh * [ones | f2 | fts]

    # DRAM I/O ------------------------------------------------------------
    xT = nc.dram_tensor("xT", [fcat, N], BF16, kind="ExternalInput").ap()
    own_xT = nc.dram_tensor("own_xT", [fcat, ROWS], BF16, kind="ExternalInput").ap()
    Wcat = nc.dram_tensor("Wcat", [fcat, wtot], BF16, kind="ExternalInput").ap()
    Wsrc = nc.dram_tensor("Wsrc", [fcat, nhp], BF16, kind="ExternalInput").ap()
    bias_out = nc.dram_tensor("bias_out", [nh, hid], F32, kind="ExternalInput").ap()
    # pre-transposed bf16 0/1 adjacency columns for this core: mT[j, i]
    mT = nc.dram_tensor("mT", [N, ROWS], BF16, kind="ExternalInput").ap()
    outT = nc.dram_tensor("outT", [nh * hid, ROWS], F32, kind="ExternalOutput").ap()

    with tile.TileContext(nc) as tc, ExitStack() as ctx:
        const = ctx.enter_context(tc.sbuf_pool(name="const", bufs=1))

        # resident constants -------------------------------------------------
        w_t = []
        for k in range(kc):
            wk = const.tile([128, wtot], BF16, tag=f"w{k}", name=f"w{k}")
            nc.sync.dma_start(wk, Wcat[k * 128 : (k + 1) * 128, :])
            w_t.append(wk)
        ws_t = []
        for k in range(kc):
            wsk = const.tile([128, nhp], BF16, tag=f"ws{k}", name=f"ws{k}")
            nc.sync.dma_start(wsk, Wsrc[k * 128 : (k + 1) * 128, :])
            ws_t.append(wsk)
        cst = const.tile([1, 128 + hid + 2], F32, tag="cst", name="cst")
        nc.gpsimd.memset(cst, 1.0)
        nc.gpsimd.memset(cst[:, 128 : 130], 0.0)
        onesrow = const.tile([1, 128], F32R, tag="onesrow", name="onesrow")
        nc.vector.tensor_copy(onesrow, cst[:, 0:128])
        # lhsT for the reciprocal broadcast: [0, 0, 1, 1, ..., 1]
        maskh = const.tile([1, hid + 2], F32R, tag="maskh", name="maskh")
        nc.vector.tensor_copy(maskh, cst[:, 128 : 128 + hid + 2])
        ident = const.tile([128, 128], F32, tag="ident", name="ident")
        from concourse.masks import make_identity

        make_identity(nc, ident)
        bpp = []
        for h in range(nh):
            bt = const.tile([hid + 2, 1], F32, tag=f"bpp{h}", name=f"bpp{h}")
            nc.gpsimd.memset(bt, 0.0)
            nc.sync.dma_start(
                bt[2 : hid + 2, :], bias_out[h : h + 1, :].rearrange("a b -> b a")
            )
            bpp.append(bt)

        # per-chunk stationary blocks: [ones | f2 | fts] per head, bf16
        fts_all = const.tile([128, NCHUNK * blk], BF16, tag="fts", name="fts_all")
        fview = fts_all.rearrange("p (c h x) -> p c h x", c=NCHUNK, h=nh)
        nc.gpsimd.memset(fview[:, :, :, 0], 1.0)  # ones columns, strided
        E2_all = const.tile([128, NCHUNK * nh], F32, tag="e2", name="E2_all")
        E2p_all = const.tile([128, NCHUNK * nh], F32, tag="e2p", name="E2p_all")
        nE2p_all = const.tile([128, NCHUNK * nh], F32, tag="ne2p", name="nE2p_all")
        f1col = const.tile([128, 8 * nhp], F32, tag="f1col", name="f1col")
        r1row = []
        for h in range(nh):
            fr = const.tile([1, ROWS], F32R, tag=f"r1row{h}", name=f"r1row{h}")
            r1row.append(fr)
        r1b = []
        for h in range(nh):
            fb = const.tile([128, ROWS], BF16, tag=f"r1b{h}", name=f"r1b{h}")
            r1b.append(fb)

        def _one_pass():
            # ---- prologue: f1 -> exp(0.8 f1) own rows; fts = x @ Wcat -------
            with tc.psum_pool(name="pro", bufs=4) as pp, tc.sbuf_pool(
                name="pro_sb", bufs=2
            ) as ps:
                # own-row f1 columns first: the r1b chain is the longest pole
                sog = []
                for k in range(kc):
                    sg2 = ps.tile([128, 1024], BF16, tag=f"so{k}", name=f"so{k}")
                    (nc.gpsimd if k % 2 == 0 else nc.sync).dma_start(
                        sg2, own_xT[k * 128 : (k + 1) * 128, :]
                    )
                    sog.append(sg2)
                for r in range(8):
                    pf1 = pp.tile([128, nhp], F32, tag="f1", name="pf1")
                    for k in range(kc):
                        nc.tensor.matmul(
                            pf1,
                            lhsT=sog[k][:, r * 128 : (r + 1) * 128],
                            rhs=ws_t[k],
                            start=(k == 0),
                            stop=(k == kc - 1),
                        )
                    nc.vector.tensor_copy(f1col[:, r * nhp : (r + 1) * nhp], pf1)
                # r1row[h][i] = exp(0.8 (f1 + B)) via per-column PE transposes
                for r in range(8):
                    for h in range(nh):
                        pt = pp.tile([1, 128], F32, tag="f1", name="pt")
                        nc.tensor.transpose(
                            pt, f1col[:, r * nhp + h : r * nhp + h + 1], ident
                        )
                        with nc.allow_low_precision(reason="f32r broadcast row"):
                            nc.scalar.activation(
                                r1row[h][:, r * 128 : (r + 1) * 128],
                                pt,
                                EXP,
                                scale=1.0 - alpha,
                                bias=(1.0 - alpha) * Bvals[h],
                            )
                for h in range(nh):
                    # broadcast r1row over the 128 partitions via PE, cast bf16
                    for half in range(ROWS // 512):
                        pb = pp.tile([128, 512], F32, tag="f1", name="pb")
                        nc.tensor.matmul(
                            pb,
                            lhsT=onesrow,
                            rhs=r1row[h][:, half * 512 : (half + 1) * 512],
                            start=True,
                            stop=True,
                        )
                        nc.scalar.copy(r1b[h][:, half * 512 : (half + 1) * 512], pb)

                for jc in range(NCHUNK):
                    jg, jr = divmod(jc, 8)
                    if jr == 0:
                        sqg = []
                        for k in range(kc):
                            sg = ps.tile([128, 1024], BF16, tag=f"sq{k}", name=f"sq{k}")
                            # split the x stream across both DGE paths
                            (nc.gpsimd if (jg + k) % 2 == 0 else nc.sync).dma_start(
                                sg,
                                xT[k * 128 : (k + 1) * 128, jg * 1024 : (jg + 1) * 1024],
                            )
                            sqg.append(sg)
                    pf = pp.tile([128, wtot], F32, tag="ps", name="pf")
                    for k in range(kc):
                        nc.tensor.matmul(
                            pf,
                            lhsT=sqg[k][:, jr * 128 : (jr + 1) * 128],
                            rhs=w_t[k],
                            start=(k == 0),
                            stop=(k == kc - 1),
                        )
                    # [f2 | fts] cols -> block cols 1..hid+2 in one copy,
                    # rotated across ACT/DVE/Pool so no single engine serializes
                    cpeng = (nc.scalar.copy, nc.vector.tensor_copy)[jc % 2]
                    cpeng(
                        fview[:, jc, :, 1 : hid + 2],
                        pf[:, 0 : nh * wid].rearrange("p (h x) -> p h x", h=nh),
                    )
                    if jr == 7:
                        # per-group exp so the sweep can start after group 0
                        nc.scalar.activation(
                            E2_all[:, jg * 8 * nh : (jg + 1) * 8 * nh],
                            fview[:, jg * 8 : (jg + 1) * 8, :, 1],
                            EXP,
                            scale=1.0,
                        )
                        nc.scalar.activation(
                            E2p_all[:, jg * 8 * nh : (jg + 1) * 8 * nh],
                            fview[:, jg * 8 : (jg + 1) * 8, :, 1],
                            EXP,
                            scale=alpha,
                        )
                        nc.vector.tensor_scalar_mul(
                            nE2p_all[:, jg * 8 * nh : (jg + 1) * 8 * nh],
                            E2p_all[:, jg * 8 * nh : (jg + 1) * 8 * nh],
                            -1.0,
                        )

            if "sweep" not in phases:
                return
            # ---- attention sweep over source chunks -------------------------
            with tc.psum_pool(name="acc", bufs=1) as ap_, tc.sbuf_pool(
                name="sw", bufs=3
            ) as sw, tc.sbuf_pool(name="ep", bufs=2) as ep:
                accs = []
                for i in range(2 * nh):
                    a = ap_.tile([hid + 2, 512], F32, tag=f"acc{i}", name=f"acc{i}", bufs=1)
                    accs.append(a)
                pend = []  # GpSimd-path matmuls deferred by one chunk
                for jc in range(NCHUNK):
                    mt = sw.tile([128, ROWS], BF16, tag="mt", name="mt", bufs=mt_bufs)
                    # alternate DGE paths so neither descriptor engine serializes
                    (nc.gpsimd if jc % 2 == 0 else nc.sync).dma_start(
                        mt, mT[jc * 128 : (jc + 1) * 128, :]
                    )
                    # GpSimd takes the tail heads on interior chunks, so chunk 0
                    # (start) and the last chunk (stop) stay on the DVE path
                    if 0 < jc < NCHUNK - 1 and pool_pat:
                        if nh == 1:
                            mod = pool_pat[0] if pool_pat else 0
                            npool = 1 if (mod and jc % mod == 2) else 0
                        else:
                            npool = min(pool_pat[jc % len(pool_pat)], nh - 1)
                    else:
                        npool = 0
                    ndve = nh - npool
                    # on interior chunks one head's max() runs on ACT as a
                    # relu; its dropped  + exp(0.2 f2)  branch is restored in
                    # PSUM by an extra PE matmul of the raw mask against the
                    # E2p-scaled stationary block (exact decomposition)
                    ah = nh - npool - 1 if (act_assist and 0 < jc < NCHUNK - 1) else -1
                    u4 = sw.tile([128, nh * ROWS], BF16, tag="u4", name="u4", bufs=ut_bufs)
                    e4 = sw.tile([128, nh * ROWS], BF16, tag="e4", name="e4", bufs=et_bufs)
                    for h in range(nh):
                        u = jc * nh + h
                        if h == ah:
                            nc.scalar.activation(
                                u4[:, h * ROWS : (h + 1) * ROWS],
                                r1b[h],
                                RELU,
                                bias=nE2p_all[:, u : u + 1],
                                scale=E2_all[:, u : u + 1],
                            )
                            continue
                        # u = max(exp(0.8 f1)[i] * exp(f2)[j], exp(0.2 f2)[j])
                        nc.vector.tensor_scalar(
                            u4[:, h * ROWS : (h + 1) * ROWS],
                            r1b[h],
                            E2_all[:, u : u + 1],
                            E2p_all[:, u : u + 1],
                            op0=MULT,
                            op1=MAX,
                        )
                    # masked exp: one batched DVE multiply for the DVE heads
                    if ndve:
                        mtb = mt.unsqueeze(1).broadcast_to([128, ndve, ROWS])
                        nc.vector.tensor_tensor(
                            e4[:, : ndve * ROWS].rearrange("p (h i) -> p h i", h=ndve),
                            u4[:, : ndve * ROWS].rearrange("p (h i) -> p h i", h=ndve),
                            mtb,
                            op=MULT,
                        )
                    for h in range(ndve, nh):
                        nc.gpsimd.tensor_mul(
                            e4[:, h * ROWS : (h + 1) * ROWS],
                            u4[:, h * ROWS : (h + 1) * ROWS],
                            mt,
                        )
                    # deferred GpSimd matmuls from the previous chunk first
                    for pacc, plhs, prhs in pend:
                        nc.tensor.matmul(pacc, lhsT=plhs, rhs=prhs, start=False, stop=False)
                    pend = []
                    if ah >= 0:
                        u = jc * nh + ah
                        fsc = sw.tile([128, blk // nh], BF16, tag="fsc", name="fsc", bufs=3)
                        nc.vector.tensor_scalar_mul(
                            fsc, fview[:, jc, ah, :], E2p_all[:, u : u + 1]
                        )
                    for h in range(nh):
                        lhs = fview[:, jc, h, :]
                        for half in range(2):
                            rhs = e4[:, h * ROWS + half * 512 : h * ROWS + (half + 1) * 512]
                            if h >= ndve:
                                pend.append((accs[2 * h + half], lhs, rhs))
                            else:
                                nc.tensor.matmul(
                                    accs[2 * h + half],
                                    lhsT=lhs,
                                    rhs=rhs,
                                    start=(jc == 0),
                                    stop=(jc == NCHUNK - 1),
                                )
                    if ah >= 0:
                        for half in range(2):
                            nc.tensor.matmul(
                                accs[2 * ah + half],
                                lhsT=fsc,
                                rhs=mt[:, half * 512 : (half + 1) * 512],
                                start=False,
                                stop=False,
                            )

                # ---- epilogue: normalize (+bias, +ELU), store h^T -----------
                for h in range(nh if "ep" in phases else 0):
                    v = ep.tile([hid + 2, ROWS], F32, tag="v", name="v")
                    for half in range(2):
                        nc.scalar.copy(
                            v[:, half * 512 : (half + 1) * 512], accs[2 * h + half]
                        )
                    rc = ep.tile([1, ROWS], F32R, tag="rc", name="rc")
                    with nc.allow_low_precision(reason="f32r out of reciprocal"):
                        nc.vector.reciprocal(rc, v[0:1, :])
                    t = ep.tile([hid + 2, ROWS], F32, tag="t", name="t")
                    for half in range(2):
                        pb2 = ap_.tile(
                            [hid + 2, 512], F32, tag=f"acc{2 * h + half}", name="pb2", bufs=1
                        )
                        nc.tensor.matmul(
                            pb2,
                            lhsT=maskh,
                            rhs=rc[:, half * 512 : (half + 1) * 512],
                            start=True,
                            stop=True,
                        )
                        nc.vector.tensor_tensor(
                            t[:, half * 512 : (half + 1) * 512],
                            v[:, half * 512 : (half + 1) * 512],
                            pb2,
                            op=MULT,
                        )
                    # rows 0-1 carry harmless junk through the tail ops
                    if elu:
                        m_ = ep.tile([hid + 2, ROWS], F32, tag="m_", name="m_")
                        nc.vector.tensor_scalar(m_, t, bpp[h], 0.0, op0=ADD, op1=MIN)
                        r_ = ep.tile([hid + 2, ROWS], F32, tag="r_", name="r_")
                        nc.vector.tensor_scalar(r_, t, bpp[h], 0.0, op0=ADD, op1=MAX)
                        e2 = ep.tile([hid + 2, ROWS], F32, tag="e2t", name="e2t")
                        nc.scalar.activation(e2, m_, EXP)
                        o_ = ep.tile([hid + 2, ROWS], F32, tag="o_", name="o_")
                        nc.vector._custom_dve(ata, out=o_, in0=e2, in1=r_, s0=1.0, s1=-1.0)
                    else:
                        o_ = ep.tile([hid + 2, ROWS], F32, tag="o_", name="o_")
                        nc.vector.tensor_scalar(o_, t, bpp[h], None, op0=ADD)
                    nc.sync.dma_start(outT[h * hid : (h + 1) * hid, :], o_[2 : hid + 2, :])

        for _rep in range(repeat):
            _one_pass()

    nc.compile()
    return nc


_BUILD_CACHE: dict = {}


def _get_layer(key, *args):
    if key not in _BUILD_CACHE:
        _BUILD_CACHE[key] = _build_layer(*args)
    return _BUILD_CACHE[key]


def _make_wcat(W, a_dst, nh):
    """Per head [W @ a_dst | W], concat over heads, zero-padded to mult of 8."""
    cat = np.concatenate(
        [np.concatenate([W[h] @ a_dst[h], W[h]], axis=1) for h in range(nh)],
        axis=1,
    ).astype(np.float32)
    pad = -8 * (-cat.shape[1] // 8) - cat.shape[1]
    if pad:
        cat = np.concatenate([cat, np.zeros((cat.shape[0], pad), np.float32)], axis=1)
    return cat.astype(ml_dtypes.bfloat16)


def kernel(
    seq,
    bias_mat,
    W1,
    a1_src,
    a1_dst,
    b1_src,
    b1_dst,
    bias1,
    W2,
    a2_src,
    a2_dst,
    b2_src,
    b2_dst,
    bias2,
):
    seq = np.asarray(seq, np.float32)
    bias_mat = np.asarray(bias_mat, np.float32)
    W1, W2 = np.asarray(W1, np.float32), np.asarray(W2, np.float32)
    a1_src, a1_dst = np.asarray(a1_src, np.float32), np.asarray(a1_dst, np.float32)
    a2_src, a2_dst = np.asarray(a2_src, np.float32), np.asarray(a2_dst, np.float32)
    bias1, bias2 = np.asarray(bias1, np.float32), np.asarray(bias2, np.float32)

    x = seq[0]  # [N, F_IN]
    xT = np.ascontiguousarray(x.T).astype(ml_dtypes.bfloat16)  # [F_IN, N]
    # per-core transposed 0/1 bf16 mask slabs: mT_c[j, i] = (bias[c*ROWS+i, j] == 0)
    mTs = [
        np.ascontiguousarray(
            (bias_mat[0, c * ROWS : (c + 1) * ROWS, :] == 0.0).T
        ).astype(ml_dtypes.bfloat16)
        for c in range(NCORES)
    ]
    W1cat = _make_wcat(W1, a1_dst, NH1)
    W1s = np.concatenate([W1[h] @ a1_src[h] for h in range(NH1)], axis=1).astype(
        ml_dtypes.bfloat16
    )  # [256, 4]
    B1 = tuple(float(b1_src[h, 0] + b1_dst[h, 0]) for h in range(NH1))

    nc1 = _get_layer(("L1", B1), NH1, HID, F_IN, B1, ALPHA, True)
    in_maps = []
    for c in range(NCORES):
        in_maps.append(
            {
                "xT": xT,
                "own_xT": np.ascontiguousarray(xT[:, c * ROWS : (c + 1) * ROWS]),
                "Wcat": W1cat,
                "Wsrc": W1s,
                "bias_out": bias1,
                "mT": mTs[c],
            }
        )
    res1 = bass_utils.run_bass_kernel_spmd(nc1, in_maps, core_ids=list(range(NCORES)))
    h1T = np.concatenate([r["outT"] for r in res1.results], axis=1).astype(
        ml_dtypes.bfloat16
    )  # [256, 8192]

    W2cat = _make_wcat(W2, a2_dst, 1)
    W2s = np.concatenate(
        [W2[0] @ a2_src[0], np.zeros((NH1 * HID, 3), np.float32)], axis=1
    ).astype(ml_dtypes.bfloat16)
    B2 = (float(b2_src[0, 0] + b2_dst[0, 0]),)

    nc2 = _get_layer(("L2", B2), 1, NCLS, NH1 * HID, B2, ALPHA, False)
    in_maps2 = []
    for c in range(NCORES):
        in_maps2.append(
            {
                "xT": h1T,
                "own_xT": np.ascontiguousarray(h1T[:, c * ROWS : (c + 1) * ROWS]),
                "Wcat": W2cat,
                "Wsrc": W2s,
                "bias_out": bias2,
                "mT": mTs[c],
            }
        )
    res2 = bass_utils.run_bass_kernel_spmd(nc2, in_maps2, core_ids=list(range(NCORES)))
    outT = np.concatenate([r["outT"] for r in res2.results], axis=1)  # [16, 8192]
    return np.ascontiguousarray(outT.T)[None].astype(np.float32)  # [1, 8192, 16]


# revision 29
# speedup vs baseline: 2.1125x; 1.3194x over previous
"""GAT (2-layer graph attention) Trainium2 Bass kernel, 8-core row-parallel.

Strategy
--------
Shard the destination-node dimension N=8192 across 8 cores (1024 rows each).
Attention tiles are computed TRANSPOSED, [j=128 src partitions, i=1024 free].

Key identity: exp is monotone, so for z = f1[i] + f2[j],
    exp(leaky_relu(z)) = max(exp(z), exp(0.2 z))
and both branches are rank-1 separable. Dividing by the i-only factor
exp(0.2 f1[i]) (which cancels in the softmax normalization) gives
    e~[j,i] = m[j,i] * max( exp(0.8 f1[i]) * exp(f2[j]), exp(0.2 f2[j]) )
with m the 0/1 adjacency mask. Per (chunk, head) unit this is ONE stock
tensor_scalar (bf16, 4x DVE perf mode; scalars exp(f2), exp(0.2 f2) ride the
per-partition slots) plus ONE bf16 mask multiply (DVE 2x, a slice of units
offloaded to GpSimd) - no table exp over the [N, N/8] attention matrix at all.

The PV matmul needs no on-chip transposes: [ones | f2 | fts] is the
stationary operand (bf16), the masked-exp tile is the moving operand,
accumulated over all 64 source chunks in PSUM; the softmax row-sum falls out
of the same matmul via the ones column, so the division is applied to the
[hid+2, 1024] result, not the [8192, 1024] attention matrix.

The adjacency mask is transposed and cast to bf16 0/1 on the host as part of
sharding; each core streams its own [8192, 1024] column slab once per layer,
alternating between the two DMA descriptor-generation paths.
Layer-1 output h1^T is produced directly in the [feature, node] layout
layer 2 needs as its stationary operand; it is gathered on the host
between the two launches.
"""

import sys

if "/opt/trn_rl_repo" not in sys.path:
    sys.path.insert(0, "/opt/trn_rl_repo")

from contextlib import ExitStack

import ml_dtypes
import numpy as np

import concourse.bass as bass
import concourse.mybir as mybir
import concourse.tile as tile
from concourse import bacc, bass_utils
from concourse import dve_ops

F32 = mybir.dt.float32
F32R = mybir.dt.float32r
BF16 = mybir.dt.bfloat16
EXP = mybir.ActivationFunctionType.Exp
RELU = mybir.ActivationFunctionType.Relu
ADD = mybir.AluOpType.add
MIN = mybir.AluOpType.min
MAX = mybir.AluOpType.max
MULT = mybir.AluOpType.mult

N, F_IN, HID, NH1, NCLS = 8192, 256, 64, 4, 16
NCORES = 8
ROWS = N // NCORES  # 1024 destination rows per core
NCHUNK = N // 128  # 64 source-node chunks
ALPHA = 0.2


def _build_layer(nh, hid, fcat, Bvals, alpha, elu, repeat=1, pool_pat=None,
                 ut_bufs=3, et_bufs=4, mt_bufs=6, act_assist=None,
                 phases=("pro", "sweep", "ep")):
    """Build one SPMD launch (layer). Returns the compiled Bacc.

    nh:   number of heads (4 or 1)
    hid:  per-head output width (64 or 16)
    fcat: input feature dim (256 both layers)
    Bvals: per-head b_src+b_dst floats, folded into f1
    elu:  apply ELU activation to the normalized output
    """
    if pool_pat is None:
        pool_pat = (1,) if nh > 1 else (6,)
    if act_assist is None:
        act_assist = 2 if nh > 1 else 0
    act_assist = int(act_assist)
    ata = next(o for o in dve_ops.OPS if o.name == "AFFINE_THEN_ADD")
    nc = bacc.Bacc("TRN2", target_bir_lowering=False, debug=False, num_devices=1)
    kc = fcat // 128  # contraction chunks for the feature matmul
    nhp = max(nh, 4)  # f32r matmul needs moving free dim > 1; pad Wsrc
    wid = hid + 1  # [W@a_dst | W] per head
    wtot = -8 * (-(nh * wid) // 8)  # f32r moving width, padded to mult of 8
    blk = nh * (hid + 2)  # per-chunk stationary storage: nh * [ones | f2 | fts]

    # DRAM I/O ------------------------------------------------------------
    xT = nc.dram_tensor("xT", [fcat, N], BF16, kind="ExternalInput").ap()
    own_xT = nc.dram_tensor("own_xT", [fcat, ROWS], BF16, kind="ExternalInput").ap()
    Wcat = nc.dram_tensor("Wcat", [fcat, wtot], BF16, kind="ExternalInput").ap()
    Wsrc = nc.dram_tensor("Wsrc", [fcat, nhp], BF16, kind="ExternalInput").ap()
    bias_out = nc.dram_tensor("bias_out", [nh, hid], F32, kind="ExternalInput").ap()
    # pre-transposed bf16 0/1 adjacency columns for this core: mT[j, i]
    mT = nc.dram_tensor("mT", [N, ROWS], BF16, kind="ExternalInput").ap()
    outT = nc.dram_tensor("outT", [nh * hid, ROWS], F32, kind="ExternalOutput").ap()

    with tile.TileContext(nc) as tc, ExitStack() as ctx:
        const = ctx.enter_context(tc.sbuf_pool(name="const", bufs=1))

        # resident constants -------------------------------------------------
        w_t = []
        for k in range(kc):
            wk = const.tile([128, wtot], BF16, tag=f"w{k}", name=f"w{k}")
            nc.sync.dma_start(wk, Wcat[k * 128 : (k + 1) * 128, :])
            w_t.append(wk)
        ws_t = []
        for k in range(kc):
            wsk = const.tile([128, nhp], BF16, tag=f"ws{k}", name=f"ws{k}")
            nc.sync.dma_start(wsk, Wsrc[k * 128 : (k + 1) * 128, :])
            ws_t.append(wsk)
        cst = const.tile([1, 128 + hid + 2], F32, tag="cst", name="cst")
        nc.gpsimd.memset(cst, 1.0)
        nc.gpsimd.memset(cst[:, 128 : 130], 0.0)
        onesrow = const.tile([1, 128], F32R, tag="onesrow", name="onesrow")
        nc.vector.tensor_copy(onesrow, cst[:, 0:128])
        # lhsT for the reciprocal broadcast: [0, 0, 1, 1, ..., 1]
        maskh = const.tile([1, hid + 2], F32R, tag="maskh", name="maskh")
        nc.vector.tensor_copy(maskh, cst[:, 128 : 128 + hid + 2])
        ident = const.tile([128, 128], F32, tag="ident", name="ident")
        from concourse.masks import make_identity

        make_identity(nc, ident)
        bpp = []
        for h in range(nh):
            bt = const.tile([hid + 2, 1], F32, tag=f"bpp{h}", name=f"bpp{h}")
            nc.gpsimd.memset(bt, 0.0)
            nc.sync.dma_start(
                bt[2 : hid + 2, :], bias_out[h : h + 1, :].rearrange("a b -> b a")
            )
            bpp.append(bt)

        # per-chunk stationary blocks: [ones | f2 | fts] per head, bf16
        fts_all = const.tile([128, NCHUNK * blk], BF16, tag="fts", name="fts_all")
        fview = fts_all.rearrange("p (c h x) -> p c h x", c=NCHUNK, h=nh)
        nc.gpsimd.memset(fview[:, :, :, 0], 1.0)  # ones columns, strided
        E2_all = const.tile([128, NCHUNK * nh], F32, tag="e2", name="E2_all")
        E2p_all = const.tile([128, NCHUNK * nh], F32, tag="e2p", name="E2p_all")
        nE2p_all = const.tile([128, NCHUNK * nh], F32, tag="ne2p", name="nE2p_all")
        f1col = const.tile([128, 8 * nhp], F32, tag="f1col", name="f1col")
        r1row = []
        for h in range(nh):
            fr = const.tile([1, ROWS], F32R, tag=f"r1row{h}", name=f"r1row{h}")
            r1row.append(fr)
        r1b = []
        for h in range(nh):
            fb = const.tile([128, ROWS], BF16, tag=f"r1b{h}", name=f"r1b{h}")
            r1b.append(fb)

        def _one_pass():
            # ---- prologue: f1 -> exp(0.8 f1) own rows; fts = x @ Wcat -------
            with tc.psum_pool(name="pro", bufs=4) as pp, tc.sbuf_pool(
                name="pro_sb", bufs=2
            ) as ps:
                # own-row f1 columns first: the r1b chain is the longest pole
                sog = []
                for k in range(kc):
                    sg2 = ps.tile([128, 1024], BF16, tag=f"so{k}", name=f"so{k}")
                    (nc.gpsimd if k % 2 == 0 else nc.sync).dma_start(
                        sg2, own_xT[k * 128 : (k + 1) * 128, :]
                    )
                    sog.append(sg2)
                for r in range(8):
                    pf1 = pp.tile([128, nhp], F32, tag="f1", name="pf1")
                    for k in range(kc):
                        nc.tensor.matmul(
                            pf1,
                            lhsT=sog[k][:, r * 128 : (r + 1) * 128],
                            rhs=ws_t[k],
                            start=(k == 0),
                            stop=(k == kc - 1),
                        )
                    nc.vector.tensor_copy(f1col[:, r * nhp : (r + 1) * nhp], pf1)
                # r1row[h][i] = exp(0.8 (f1 + B)) via per-column PE transposes
                for r in range(8):
                    for h in range(nh):
                        pt = pp.tile([1, 128], F32, tag="f1", name="pt")
                        nc.tensor.transpose(
                            pt, f1col[:, r * nhp + h : r * nhp + h + 1], ident
                        )
                        with nc.allow_low_precision(reason="f32r broadcast row"):
                            nc.scalar.activation(
                                r1row[h][:, r * 128 : (r + 1) * 128],
                                pt,
                                EXP,
                                scale=1.0 - alpha,
                                bias=(1.0 - alpha) * Bvals[h],
                            )
                for h in range(nh):
                    # broadcast r1row over the 128 partitions via PE, cast bf16
                    for half in range(ROWS // 512):
                        pb = pp.tile([128, 512], F32, tag="f1", name="pb")
                        nc.tensor.matmul(
                            pb,
                            lhsT=onesrow,
                            rhs=r1row[h][:, half * 512 : (half + 1) * 512],
                            start=True,
                            stop=True,
                        )
                        nc.scalar.copy(r1b[h][:, half * 512 : (half + 1) * 512], pb)

                for jc in range(NCHUNK):
                    jg, jr = divmod(jc, 8)
                    if jr == 0:
                        sqg = []
                        for k in range(kc):
                            sg = ps.tile([128, 1024], BF16, tag=f"sq{k}", name=f"sq{k}")
                            # split the x stream across both DGE paths
                            (nc.gpsimd if (jg + k) % 2 == 0 else nc.sync).dma_start(
                                sg,
                                xT[k * 128 : (k + 1) * 128, jg * 1024 : (jg + 1) * 1024],
                            )
                            sqg.append(sg)
                    pf = pp.tile([128, wtot], F32, tag="ps", name="pf")
                    for k in range(kc):
                        nc.tensor.matmul(
                            pf,
                            lhsT=sqg[k][:, jr * 128 : (jr + 1) * 128],
                            rhs=w_t[k],
                            start=(k == 0),
                            stop=(k == kc - 1),
                        )
                    # [f2 | fts] cols -> block cols 1..hid+2 in one copy,
                    # rotated across ACT/DVE/Pool so no single engine serializes
                    cpeng = (nc.scalar.copy, nc.vector.tensor_copy)[jc % 2]
                    cpeng(
                        fview[:, jc, :, 1 : hid + 2],
                        pf[:, 0 : nh * wid].rearrange("p (h x) -> p h x", h=nh),
                    )
                    if jr == 7:
                        # per-group exp so the sweep can start after group 0
                        nc.scalar.activation(
                            E2_all[:, jg * 8 * nh : (jg + 1) * 8 * nh],
                            fview[:, jg * 8 : (jg + 1) * 8, :, 1],
                            EXP,
                            scale=1.0,
                        )
                        nc.scalar.activation(
                            E2p_all[:, jg * 8 * nh : (jg + 1) * 8 * nh],
                            fview[:, jg * 8 : (jg + 1) * 8, :, 1],
                            EXP,
                            scale=alpha,
                        )
                        nc.vector.tensor_scalar_mul(
                            nE2p_all[:, jg * 8 * nh : (jg + 1) * 8 * nh],
                            E2p_all[:, jg * 8 * nh : (jg + 1) * 8 * nh],
                            -1.0,
                        )

            if "sweep" not in phases:
                return
            # ---- attention sweep over source chunks -------------------------
            with tc.psum_pool(name="acc", bufs=1) as ap_, tc.sbuf_pool(
                name="sw", bufs=3
            ) as sw, tc.sbuf_pool(name="ep", bufs=2) as ep:
                accs = []
                for i in range(2 * nh):
                    a = ap_.tile([hid + 2, 512], F32, tag=f"acc{i}", name=f"acc{i}", bufs=1)
                    accs.append(a)
                pend = []  # GpSimd-path matmuls deferred by one chunk
                for jc in range(NCHUNK):
                    mt = sw.tile([128, ROWS], BF16, tag="mt", name="mt", bufs=mt_bufs)
                    # alternate DGE paths so neither descriptor engine serializes
                    (nc.gpsimd if jc % 2 == 0 else nc.sync).dma_start(
                        mt, mT[jc * 128 : (jc + 1) * 128, :]
                    )
                    # GpSimd takes the tail heads on interior chunks, so chunk 0
                    # (start) and the last chunk (stop) stay on the DVE path
                    if 0 < jc < NCHUNK - 1 and pool_pat:
                        if nh == 1:
                            mod = pool_pat[0] if pool_pat else 0
                            npool = 1 if (mod and jc % mod == 2) else 0
                        else:
                            npool = min(pool_pat[jc % len(pool_pat)], nh - 1)
                    else:
                        npool = 0
                    ndve = nh - npool
                    # on interior chunks the last act_assist DVE-batched heads'
                    # max() runs on ACT as a relu; the dropped  + exp(0.2 f2)
                    # branch is restored in PSUM by an extra PE matmul of the
                    # raw mask against the E2p-scaled block (exact decomposition)
                    if act_assist and 0 < jc < NCHUNK - 1:
                        aset = set(range(max(ndve - act_assist, 0), ndve))
                    else:
                        aset = set()
                    u4 = sw.tile([128, nh * ROWS], BF16, tag="u4", name="u4", bufs=ut_bufs)
                    e4 = sw.tile([128, nh * ROWS], BF16, tag="e4", name="e4", bufs=et_bufs)
                    for h in range(nh):
                        u = jc * nh + h
                        if h in aset:
                            nc.scalar.activation(
                                u4[:, h * ROWS : (h + 1) * ROWS],
                                r1b[h],
                                RELU,
                                bias=nE2p_all[:, u : u + 1],
                                scale=E2_all[:, u : u + 1],
                            )
                            continue
                        # u = max(exp(0.8 f1)[i] * exp(f2)[j], exp(0.2 f2)[j])
                        nc.vector.tensor_scalar(
                            u4[:, h * ROWS : (h + 1) * ROWS],
                            r1b[h],
                            E2_all[:, u : u + 1],
                            E2p_all[:, u : u + 1],
                            op0=MULT,
                            op1=MAX,
                        )
                    # masked exp: one batched DVE multiply for the DVE heads
                    if ndve:
                        mtb = mt.unsqueeze(1).broadcast_to([128, ndve, ROWS])
                        nc.vector.tensor_tensor(
                            e4[:, : ndve * ROWS].rearrange("p (h i) -> p h i", h=ndve),
                            u4[:, : ndve * ROWS].rearrange("p (h i) -> p h i", h=ndve),
                            mtb,
                            op=MULT,
                        )
                    for h in range(ndve, nh):
                        nc.gpsimd.tensor_mul(
                            e4[:, h * ROWS : (h + 1) * ROWS],
                            u4[:, h * ROWS : (h + 1) * ROWS],
                            mt,
                        )
                    # deferred GpSimd matmuls from the previous chunk first
                    for pacc, plhs, prhs in pend:
                        nc.tensor.matmul(pacc, lhsT=plhs, rhs=prhs, start=False, stop=False)
                    pend = []
                    fscs = {}
                    for ah in aset:
                        u = jc * nh + ah
                        fsc = sw.tile(
                            [128, blk // nh], BF16, tag=f"fsc{ah}", name="fsc", bufs=3
                        )
                        nc.vector.tensor_scalar_mul(
                            fsc, fview[:, jc, ah, :], E2p_all[:, u : u + 1]
                        )
                        fscs[ah] = fsc
                    for h in range(nh):
                        lhs = fview[:, jc, h, :]
                        for half in range(2):
                            rhs = e4[:, h * ROWS + half * 512 : h * ROWS + (half + 1) * 512]
                            if h >= ndve:
                                pend.append((accs[2 * h + half], lhs, rhs))
                            else:
                                nc.tensor.matmul(
                                    accs[2 * h + half],
                                    lhsT=lhs,
                                    rhs=rhs,
                                    start=(jc == 0),
                                    stop=(jc == NCHUNK - 1),
                                )
                    for ah in aset:
                        for half in range(2):
                            nc.tensor.matmul(
                                accs[2 * ah + half],
                                lhsT=fscs[ah],
                                rhs=mt[:, half * 512 : (half + 1) * 512],
                                start=False,
                                stop=False,
                            )

                # ---- epilogue: normalize (+bias, +ELU), store h^T -----------
                for h in range(nh if "ep" in phases else 0):
                    v = ep.tile([hid + 2, ROWS], F32, tag="v", name="v")
                    for half in range(2):
                        nc.scalar.copy(
                            v[:, half * 512 : (half + 1) * 512], accs[2 * h + half]
                        )
                    rc = ep.tile([1, ROWS], F32R, tag="rc", name="rc")
                    with nc.allow_low_precision(reason="f32r out of reciprocal"):
                        nc.vector.reciprocal(rc, v[0:1, :])
                    t = ep.tile([hid + 2, ROWS], F32, tag="t", name="t")
                    for half in range(2):
                        pb2 = ap_.tile(
                            [hid + 2, 512], F32, tag=f"acc{2 * h + half}", name="pb2", bufs=1
                        )
                        nc.tensor.matmul(
                            pb2,
                            lhsT=maskh,
                            rhs=rc[:, half * 512 : (half + 1) * 512],
                            start=True,
                            stop=True,
                        )
                        nc.vector.tensor_tensor(
                            t[:, half * 512 : (half + 1) * 512],
                            v[:, half * 512 : (half + 1) * 512],
                            pb2,
                            op=MULT,
                        )
                    # rows 0-1 carry harmless junk through the tail ops
                    if elu:
                        m_ = ep.tile([hid + 2, ROWS], F32, tag="m_", name="m_")
                        nc.vector.tensor_scalar(m_, t, bpp[h], 0.0, op0=ADD, op1=MIN)
                        r_ = ep.tile([hid + 2, ROWS], F32, tag="r_", name="r_")
                        nc.vector.tensor_scalar(r_, t, bpp[h], 0.0, op0=ADD, op1=MAX)
                        e2 = ep.tile([hid + 2, ROWS], F32, tag="e2t", name="e2t")
                        nc.scalar.activation(e2, m_, EXP)
                        o_ = ep.tile([hid + 2, ROWS], F32, tag="o_", name="o_")
                        nc.vector._custom_dve(ata, out=o_, in0=e2, in1=r_, s0=1.0, s1=-1.0)
                    else:
                        o_ = ep.tile([hid + 2, ROWS], F32, tag="o_", name="o_")
                        nc.vector.tensor_scalar(o_, t, bpp[h], None, op0=ADD)
                    nc.sync.dma_start(outT[h * hid : (h + 1) * hid, :], o_[2 : hid + 2, :])

        for _rep in range(repeat):
            _one_pass()

    nc.compile()
    return nc


_BUILD_CACHE: dict = {}


def _get_layer(key, *args):
    if key not in _BUILD_CACHE:
        _BUILD_CACHE[key] = _build_layer(*args)
    return _BUILD_CACHE[key]


def _make_wcat(W, a_dst, nh):
    """Per head [W @ a_dst | W], concat over heads, zero-padded to mult of 8."""
    cat = np.concatenate(
        [np.concatenate([W[h] @ a_dst[h], W[h]], axis=1) for h in range(nh)],
        axis=1,
    ).astype(np.float32)
    pad = -8 * (-cat.shape[1] // 8) - cat.shape[1]
    if pad:
        cat = np.concatenate([cat, np.zeros((cat.shape[0], pad), np.float32)], axis=1)
    return cat.astype(ml_dtypes.bfloat16)


def kernel(
    seq,
    bias_mat,
    W1,
    a1_src,
    a1_dst,
    b1_src,
    b1_dst,
    bias1,
    W2,
    a2_src,
    a2_dst,
    b2_src,
    b2_dst,
    bias2,
):
    seq = np.asarray(seq, np.float32)
    bias_mat = np.asarray(bias_mat, np.float32)
    W1, W2 = np.asarray(W1, np.float32), np.asarray(W2, np.float32)
    a1_src, a1_dst = np.asarray(a1_src, np.float32), np.asarray(a1_dst, np.float32)
    a2_src, a2_dst = np.asarray(a2_src, np.float32), np.asarray(a2_dst, np.float32)
    bias1, bias2 = np.asarray(bias1, np.float32), np.asarray(bias2, np.float32)

    x = seq[0]  # [N, F_IN]
    xT = np.ascontiguousarray(x.T).astype(ml_dtypes.bfloat16)  # [F_IN, N]
    # per-core transposed 0/1 bf16 mask slabs: mT_c[j, i] = (bias[c*ROWS+i, j] == 0)
    mTs = [
        np.ascontiguousarray(
            (bias_mat[0, c * ROWS : (c + 1) * ROWS, :] == 0.0).T
        ).astype(ml_dtypes.bfloat16)
        for c in range(NCORES)
    ]
    W1cat = _make_wcat(W1, a1_dst, NH1)
    W1s = np.concatenate([W1[h] @ a1_src[h] for h in range(NH1)], axis=1).astype(
        ml_dtypes.bfloat16
    )  # [256, 4]
    B1 = tuple(float(b1_src[h, 0] + b1_dst[h, 0]) for h in range(NH1))

    nc1 = _get_layer(("L1", B1), NH1, HID, F_IN, B1, ALPHA, True)
    in_maps = []
    for c in range(NCORES):
        in_maps.append(
            {
                "xT": xT,
                "own_xT": np.ascontiguousarray(xT[:, c * ROWS : (c + 1) * ROWS]),
                "Wcat": W1cat,
                "Wsrc": W1s,
                "bias_out": bias1,
                "mT": mTs[c],
            }
        )
    res1 = bass_utils.run_bass_kernel_spmd(nc1, in_maps, core_ids=list(range(NCORES)))
    h1T = np.concatenate([r["outT"] for r in res1.results], axis=1).astype(
        ml_dtypes.bfloat16
    )  # [256, 8192]

    W2cat = _make_wcat(W2, a2_dst, 1)
    W2s = np.concatenate(
        [W2[0] @ a2_src[0], np.zeros((NH1 * HID, 3), np.float32)], axis=1
    ).astype(ml_dtypes.bfloat16)
    B2 = (float(b2_src[0, 0] + b2_dst[0, 0]),)

    nc2 = _get_layer(("L2", B2), 1, NCLS, NH1 * HID, B2, ALPHA, False)
    in_maps2 = []
    for c in range(NCORES):
        in_maps2.append(
            {
                "xT": h1T,
                "own_xT": np.ascontiguousarray(h1T[:, c * ROWS : (c + 1) * ROWS]),
                "Wcat": W2cat,
                "Wsrc": W2s,
                "bias_out": bias2,
                "mT": mTs[c],
            }
        )
    res2 = bass_utils.run_bass_kernel_spmd(nc2, in_maps2, core_ids=list(range(NCORES)))
    outT = np.concatenate([r["outT"] for r in res2.results], axis=1)  # [16, 8192]
    return np.ascontiguousarray(outT.T)[None].astype(np.float32)  # [1, 8192, 16]
